# revision 8
# baseline (speedup 1.0000x reference)
"""Trainium2 Bass kernel for nn_CustomLoss (CrossEntropy + binary-remap BCE).

loss = mean_i[ logsumexp(pred_i) - pred_i[t_i] ]
     + 100 * mean_i[ 1{ LUT[argmax(pred_i)] != LUT[t_i] } ]

with LUT = [0,0,1,1,1,1,1,1,0,0]  (LUT[j] = 1 iff 2 <= j <= 7).

Sharding: data-parallel over the batch axis across 8 NeuronCores; each core
returns 3 per-partition partial sums which the host folds into the scalar.

Per-core pipeline (engine assignment tuned from perfetto traces):
  DMA   : pred tiles [128, W*10] f32 (contiguous rows), packed aux gs tiles
  PE    : transpose 120-column blocks to PSUM, then rotated block-diagonal
          matmuls against exp(pred) compute per-row sum-of-exp directly in
          PSUM (10 rotations fill 120 output partitions)
  ACT   : exp (PSUM->SBUF bf16), Ln of row-sums with per-partition accumulate
  GPSIMD: outer-4 group maxes + compare (BCE path), part of the mid-6 maxes
  DVE   : custom fused op GATHER_EQ_SUM (one-hot gather + accumulate),
          custom fused op MISMATCH_XOR_SUM, part of the mid-6 maxes

The gather index and binary target are packed host-side into one f32 aux
tensor gs = +-(10*w + t), sign = binary target.
"""

import numpy as np

# ---------------------------------------------------------------- constants
N = 2_000_000
C = 10
N_CORES = 8
P = 128
W = 492                       # rows per partition per tile (multiple of 12)
TILES = 4
ROWS_PER_TILE = P * W         # 62,976
ROWS_CORE_PAD = ROWS_PER_TILE * TILES   # 251,904
ROWS_CORE = N // N_CORES      # 250,000
PAD_PER_CORE = ROWS_CORE_PAD - ROWS_CORE  # 1,904

BLK = 120                     # transpose block columns (12 rows of 10)
N_ROT = 10                    # rotations to fill 120 sum partitions
# DVE handles the mid-6 max for tiles < M6_DVE_TILES, GPSIMD for the rest
M6_DVE_TILES = 2

_CACHE = {}


# ------------------------------------------------------- custom DVE ops
def _register_custom_ops():
    """Register the two fused DVE ops (idempotent)."""
    import concourse.dve_ops as dve_ops
    from concourse.dve_spec import (
        Spec, Src0, Src1, Zero, select, eq, lower, AluOp, Idx, Bin, maxx,
    )
    from concourse.dve_uop import DveOpSpec

    def _get(name):
        for op in dve_ops.OPS:
            if op.name == name:
                return op
        return None

    def _register(name, spec):
        existing = _get(name)
        if existing is not None:
            return existing
        opcode = dve_ops._CUSTOM_DVE_ROW_BASE + len(dve_ops.OPS)
        assert opcode < 0x20, "custom DVE opcode rows exhausted"
        from concourse.dve_ops import has_src1
        shas = {}
        for ver in ("v3", "v4"):
            uops = lower(spec, ver=ver)
            tmp = DveOpSpec(name=name, opcode=opcode, uops=uops,
                            rd1_en=has_src1(spec))
            shas[ver] = tmp.sha(ver)
        op = dve_ops.DveOp(name, spec, subdim=False, uops_sha=shas)
        dve_ops.OPS.append(op)
        dve_ops._SUB_OPCODE_FOR_NAME[name] = opcode
        dve_ops.CUSTOM_DVE_SPECS[name] = spec
        return op

    # GATHER_EQ_SUM: out[k] = in0[k] if k == |in1[k]| else 0; accum = sum out
    def _gather_ref(in0, in1, s0, s1, imm2):
        p = in0.shape[0]
        f0 = np.asarray(in0, np.float32).reshape(p, -1)
        f1 = np.abs(np.asarray(in1, np.float32).reshape(p, -1))
        idx = np.arange(f0.shape[1], dtype=np.float32)[None, :]
        out = np.where(idx == f1, f0, np.float32(0.0))
        acc = out.sum(axis=1, dtype=np.float64).astype(np.float32)[:, None]
        return out.reshape(in0.shape), acc

    gather_spec = Spec(
        body=select(eq(Idx, maxx(Src1, Zero - Src1)), Src0, Zero),
        accum=AluOp.ADD,
        accum_init=Zero,
        reference=_gather_ref,
    )
    gop = _register("GATHER_EQ_SUM_ANT2", gather_spec)

    # MISMATCH_XOR_SUM: out[k] = (in0[k] != 0) XOR (in1[k] > 0); accum = sum
    def _mm_ref(in0, in1, s0, s1, imm2):
        p = in0.shape[0]
        a = np.asarray(in0, np.float32).reshape(p, -1) != 0
        b = np.asarray(in1, np.float32).reshape(p, -1) > 0
        out = (a ^ b).astype(np.float32)
        acc = out.sum(axis=1, dtype=np.float64).astype(np.float32)[:, None]
        return out.reshape(in0.shape), acc

    mm_spec = Spec(
        body=Bin(AluOp.LOGICAL_XOR, Src0, Src1 > Zero),
        accum=AluOp.ADD,
        accum_init=Zero,
        reference=_mm_ref,
    )
    mop = _register("MISMATCH_XOR_SUM_ANT2", mm_spec)
    return gop, mop


def _s_matrices():
    """Rotation r weight: S[q, r, m] = 1 iff m == 12*r + q//10, laid out
    [120 partitions, N_ROT*128 free] so smat[:, 128r:128(r+1)] is lhsT."""
    import ml_dtypes
    s = np.zeros((BLK, N_ROT, P), np.float32)
    q = np.arange(BLK)
    for r in range(N_ROT):
        s[q, r, 12 * r + q // 10] = 1.0
    return s.reshape(BLK, N_ROT * P).astype(ml_dtypes.bfloat16)


def _gp_reduce(nc, out, in_, axis, op):
    """Emit a free-dim tensor_reduce on the GPSIMD (Pool) engine.

    bass's gpsimd.tensor_reduce only allows partition-axis reductions, but
    the cayman ISA supports free-dim reduces on Pool for int dtypes
    (TENSOR_REDUCE_ARITH_OP + MAX_INT).  Used on int32-bitcast views of
    all-positive floats, whose int order equals their float order."""
    from concourse import mybir
    eng = nc.gpsimd
    return eng.add_instruction(
        mybir.InstTensorReduce(
            name=f"I-{nc.next_id()}", op=op, axis=axis,
            ins=[eng.lower_ap(in_, opt=False)],
            outs=[eng.lower_ap(out)]))


# ------------------------------------------------------------- device build
def _build_nc(tiles=TILES, w=W, m6_dve_tiles=M6_DVE_TILES):
    import concourse.bass as bass
    import concourse.tile as tile
    from concourse import bacc, mybir

    gop, mop = _register_custom_ops()
    f32 = mybir.dt.float32
    i32 = mybir.dt.int32
    bf16 = mybir.dt.bfloat16
    A = mybir.ActivationFunctionType
    X = mybir.AxisListType.X
    alu = mybir.AluOpType

    assert (w * C) % BLK == 0
    n_blk = (w * C) // BLK          # blocks per tile
    g_full = n_blk // 4             # full 4-block groups per tile
    n_extra = n_blk % 4             # leftover blocks per tile
    assert g_full <= N_ROT and tiles <= N_ROT

    nc = bacc.Bacc("TRN2", target_bir_lowering=False, debug=False,
                   num_devices=N_CORES)
    pred_d = nc.dram_tensor("pred", [tiles, P, w * C], f32,
                            kind="ExternalInput").ap()
    gs_d = nc.dram_tensor("gs", [tiles, P, w], f32,
                          kind="ExternalInput").ap()
    smat_d = nc.dram_tensor("smat", [BLK, N_ROT * P], bf16,
                            kind="ExternalInput").ap()
    ident_d = nc.dram_tensor("ident", [P, P], f32,
                             kind="ExternalInput").ap()
    out_d = nc.dram_tensor("out", [P, 3], f32, kind="ExternalOutput").ap()

    with tile.TileContext(nc) as tc:
        with (
            tc.tile_pool(name="io", bufs=2) as io,
            tc.tile_pool(name="ep", bufs=2) as ep,
            tc.tile_pool(name="mp", bufs=2) as mp,
            tc.tile_pool(name="tp", bufs=1) as tp,
            tc.tile_pool(name="cp", bufs=1) as cp,
            tc.tile_pool(name="psA", bufs=2, space="PSUM") as psA,
            tc.tile_pool(name="psS", bufs=2, space="PSUM") as psS,
            tc.tile_pool(name="psX", bufs=1, space="PSUM") as psX,
        ):
            # constants + accumulators
            smat = cp.tile([BLK, N_ROT * P], bf16)
            nc.sync.dma_start(smat[:], smat_d[:])
            ident = cp.tile([P, P], f32)
            nc.sync.dma_start(ident[:], ident_d[:])
            acc_lg = cp.tile([P, tiles + 1], f32)
            nc.gpsimd.memset(acc_lg[:], 0.0)
            bias16 = cp.tile([P, 1], f32)
            nc.gpsimd.memset(bias16[:], -16.0)
            acc_g = cp.tile([P, tiles], f32)
            acc_mm = cp.tile([P, tiles], f32)
            trash = tp.tile([P, w * C], f32)
            trash2 = tp.tile([P, w], f32)
            lnt = tp.tile([P, 512], f32)
            sums_x = None
            if n_extra:
                sums_x = psX.tile([P, P * n_extra], f32, tag="sums_x")

            for i in range(tiles):
                pt = io.tile([P, w * C], f32, tag="pred")
                nc.sync.dma_start(pt[:], pred_d[i])
                gt = io.tile([P, w], f32, tag="gs")
                nc.sync.dma_start(gt[:], gs_d[i])

                # ---------- CE: transpose blocks, exp, rotated matmul sums
                sums = psS.tile([P, 512], f32, tag="sums")
                # batches of up to 8 blocks share one PSUM staging tile
                n_batch = (n_blk + 7) // 8
                e_tiles = []
                for b in range(n_batch):
                    blo = b * 8
                    bhi = min(blo + 8, n_blk)
                    nb = bhi - blo
                    tb = psA.tile([BLK, P * 8], f32, tag="tb")
                    for j in range(nb):
                        g = blo + j
                        nc.tensor.transpose(
                            tb[:, P * j:P * (j + 1)],
                            pt[:, BLK * g:BLK * (g + 1)],
                            ident[:])
                    et = ep.tile([BLK, P * 8], bf16, tag="et")
                    nc.scalar.activation(et[:, 0:P * nb], tb[:, 0:P * nb],
                                         A.Exp, bias=bias16[0:BLK, :])
                    e_tiles.append((et, nb))

                # full 4-block groups -> rotations 0..g_full-1
                for r in range(g_full):
                    et, nb = e_tiles[(4 * r) // 8]
                    off = (4 * r) % 8
                    rhs = et[:, P * off:P * (off + 4)]
                    nc.tensor.matmul(sums[:], smat[:, P * r:P * (r + 1)],
                                     rhs, start=(r == 0),
                                     stop=(r == g_full - 1))
                # leftover blocks -> shared cross-tile sums, rotation = i
                if n_extra:
                    et, nb = e_tiles[-1]
                    off = 8 * (n_batch - 1)
                    lo = (4 * g_full) - off
                    rhs = et[:, P * lo:P * (lo + n_extra)]
                    nc.tensor.matmul(sums_x[:, 0:P * n_extra],
                                     smat[:, P * i:P * (i + 1)], rhs,
                                     start=(i == 0), stop=(i == tiles - 1),
                                     skip_group_check=True)

                np_full = 12 * g_full
                nc.scalar.activation(lnt[0:np_full, :], sums[0:np_full, :],
                                     A.Ln, accum_out=acc_lg[0:np_full,
                                                           i:i + 1])

                # ---------- BCE: group maxes.  pred is host-shifted +16 so
                # all values are positive and int32 bit order == float order;
                # Pool supports free-dim MAX_INT reduces.
                pti = pt[:].bitcast(i32)
                p3i = pti.rearrange("p (w c) -> p w c", c=C)
                p4i = pti.rearrange("p (w g e) -> p w g e", g=5, e=2)
                m6 = mp.tile([P, w], i32, tag="m6")
                if i < m6_dve_tiles:
                    nc.vector.reduce_max(m6[:], p3i[:, :, 2:8], axis=X)
                else:
                    _gp_reduce(nc, m6[:], p3i[:, :, 2:8], axis=X, op=alu.max)
                m4 = mp.tile([P, w], i32, tag="m4")
                _gp_reduce(nc, m4[:], p4i[:, :, 0:5:4, :],
                           axis=mybir.AxisListType.XY, op=alu.max)
                bp = mp.tile([P, w], f32, tag="bp")
                nc.vector.tensor_tensor(bp[:], m6[:].bitcast(f32),
                                        m4[:].bitcast(f32), op=alu.is_gt)

                # ---------- fused gather + mismatch accumulation (DVE)
                nc.vector._custom_dve(
                    gop, out=trash[:],
                    in0=pt[:].rearrange("p (w c) -> p w c", c=C),
                    in1=gt[:].unsqueeze(2).broadcast_to([P, w, C]),
                    accum_out=acc_g[:, i:i + 1])
                nc.vector._custom_dve(
                    mop, out=trash2[:], in0=bp[:], in1=gt[:],
                    accum_out=acc_mm[:, i:i + 1])

            # leftover-block sums: Ln after last tile accumulated
            if n_extra:
                np_x = 12 * tiles
                nc.scalar.activation(lnt[0:np_x, 0:P * n_extra],
                                     sums_x[0:np_x, 0:P * n_extra],
                                     A.Ln,
                                     accum_out=acc_lg[0:np_x,
                                                      tiles:tiles + 1])

            # ---------- final per-partition reductions + store
            out_t = cp.tile([P, 3], f32)
            nc.vector.reduce_sum(out_t[:, 0:1], acc_lg[:], axis=X)
            nc.vector.reduce_sum(out_t[:, 1:2], acc_g[:], axis=X)
            nc.vector.reduce_sum(out_t[:, 2:3], acc_mm[:], axis=X)
            nc.sync.dma_start(out_d[:], out_t[:])

    # Force a single activation table containing both Exp and Ln so the
    # compiler does not ping-pong ACT_TABLE_LOADs between tiles.  Table ids
    # are positional, so keep the dict shape and empty the other sets.
    import concourse.bacc as bacc_mod
    from concourse.hw_specs import get_activation_tables
    orig = get_activation_tables(nc.m.arch)
    combined = None
    for k, v in orig.items():
        if (mybir.ActivationFunctionType.Exp in v
                and mybir.ActivationFunctionType.Ln in v):
            combined = k
            break
    if combined is not None:
        patched = {k: (v if k == combined else set()) for k, v in orig.items()}
        saved = bacc_mod.get_activation_tables
        bacc_mod.get_activation_tables = lambda arch: patched
        try:
            nc.compile()
        finally:
            bacc_mod.get_activation_tables = saved
    else:
        nc.compile()
    return nc


def _get_nc():
    if "nc" not in _CACHE:
        _CACHE["nc"] = _build_nc()
    return _CACHE["nc"]


# ------------------------------------------------------------------- host
def _host_prep(pred, target):
    """Shard + pad inputs, build the packed gs aux tensor per core."""
    pred = np.ascontiguousarray(np.asarray(pred, dtype=np.float32))
    target = np.asarray(target).astype(np.int32)
    smat = _s_matrices()
    ident = np.eye(P, dtype=np.float32)

    in_maps = []
    rows = ROWS_CORE
    for c in range(N_CORES):
        pc = pred[c * rows:(c + 1) * rows] + np.float32(16.0)
        tc_ = target[c * rows:(c + 1) * rows]
        if PAD_PER_CORE:
            pc = np.concatenate(
                [pc, np.full((PAD_PER_CORE, C), 16.0, np.float32)], axis=0)
            tc_ = np.concatenate(
                [tc_, np.zeros(PAD_PER_CORE, np.int32)], axis=0)
        pc = pc.reshape(TILES, P, W * C)
        tc_ = tc_.reshape(TILES, P, W)
        w_idx = np.broadcast_to(
            np.arange(W, dtype=np.int64) * C, (TILES, P, W))
        g = (w_idx + tc_).astype(np.float32)
        bt = (tc_ >= 2) & (tc_ <= 7)
        gs = np.where(bt, g, -g).astype(np.float32)
        in_maps.append({"pred": np.ascontiguousarray(pc),
                        "gs": np.ascontiguousarray(gs),
                        "smat": smat, "ident": ident})
    return in_maps


def kernel(pred, target):
    from concourse.bass_utils import run_bass_kernel_spmd

    nc = _get_nc()
    in_maps = _host_prep(pred, target)
    res = run_bass_kernel_spmd(nc, in_maps, core_ids=list(range(N_CORES)))

    sum_lg = 0.0
    sum_g = 0.0
    sum_mm = 0.0
    for c in range(N_CORES):
        o = res.results[c]["out"].astype(np.float64)
        sum_lg += o[:, 0].sum()
        sum_g += o[:, 1].sum()
        sum_mm += o[:, 2].sum()

    # padded rows: pred' = 16 -> logsumexp = ln(10), mismatch 0.
    # the gather picks pred' = pred + 16, so subtract the shift for every
    # (real and padded) row.
    sum_lg -= N_CORES * PAD_PER_CORE * np.log(10.0)
    sum_g -= 16.0 * N_CORES * ROWS_CORE_PAD

    ce = (sum_lg - sum_g) / N
    bce = 100.0 * sum_mm / N
    return np.float32(ce + bce)


# revision 9
# speedup vs baseline: 1.4762x; 1.4762x over previous
"""Trainium2 Bass kernel for nn_CustomLoss (CrossEntropy + binary-remap BCE).

loss = mean_i[ logsumexp(pred_i) - pred_i[t_i] ]
     + 100 * mean_i[ 1{ LUT[argmax(pred_i)] != LUT[t_i] } ]

with LUT = [0,0,1,1,1,1,1,1,0,0]  (LUT[j] = 1 iff 2 <= j <= 7).

Sharding: data-parallel over the batch axis across 8 NeuronCores; each core
returns 3 per-partition partial sums which the host folds into the scalar.

Per-core pipeline (engine assignment tuned from perfetto traces):
  DMA   : pred tiles [128, W*10] f32 (contiguous rows), packed aux gs tiles
  ACT   : E = exp(pred' - 16) (one combined Exp/Ln table, no table thrash),
          Ln of the per-row sums with per-partition accumulate
  GPSIMD: per-row sum of E via a strided f32 add tree (10 -> 5 -> 2+1 -> 1)
  DVE   : mid-6 / outer-4 group max reduces + compare (BCE path),
          custom fused op GATHER_EQ_SUM (one-hot gather + accumulate),
          custom fused op MISMATCH_XOR_SUM

Host packs pred' = pred + 16 (shift absorbed by the exp bias and corrected
in the final sums) and one aux f32 tensor gs = +-(10*w + t) whose sign is
the binary target.
"""

import numpy as np

# ---------------------------------------------------------------- constants
N = 2_000_000
C = 10
N_CORES = 8
P = 128
W = 652                       # rows per partition per tile
TILES = 3
ROWS_PER_TILE = P * W         # 83,456
ROWS_CORE_PAD = ROWS_PER_TILE * TILES   # 250,368
ROWS_CORE = N // N_CORES      # 250,000
PAD_PER_CORE = ROWS_CORE_PAD - ROWS_CORE  # 368
SHIFT = 16.0

_CACHE = {}


# ------------------------------------------------------- custom DVE ops
def _register_custom_ops():
    """Register the two fused DVE ops (idempotent)."""
    import concourse.dve_ops as dve_ops
    from concourse.dve_spec import (
        Spec, Src0, Src1, Zero, select, eq, lower, AluOp, Idx, Bin, maxx,
    )
    from concourse.dve_uop import DveOpSpec

    def _get(name):
        for op in dve_ops.OPS:
            if op.name == name:
                return op
        return None

    def _register(name, spec):
        existing = _get(name)
        if existing is not None:
            return existing
        opcode = dve_ops._CUSTOM_DVE_ROW_BASE + len(dve_ops.OPS)
        assert opcode < 0x20, "custom DVE opcode rows exhausted"
        from concourse.dve_ops import has_src1
        shas = {}
        for ver in ("v3", "v4"):
            uops = lower(spec, ver=ver)
            tmp = DveOpSpec(name=name, opcode=opcode, uops=uops,
                            rd1_en=has_src1(spec))
            shas[ver] = tmp.sha(ver)
        op = dve_ops.DveOp(name, spec, subdim=False, uops_sha=shas)
        dve_ops.OPS.append(op)
        dve_ops._SUB_OPCODE_FOR_NAME[name] = opcode
        dve_ops.CUSTOM_DVE_SPECS[name] = spec
        return op

    # GATHER_EQ_SUM: out[k] = in0[k] if k == |in1[k]| else 0; accum = sum out
    def _gather_ref(in0, in1, s0, s1, imm2):
        p = in0.shape[0]
        f0 = np.asarray(in0, np.float32).reshape(p, -1)
        f1 = np.abs(np.asarray(in1, np.float32).reshape(p, -1))
        idx = np.arange(f0.shape[1], dtype=np.float32)[None, :]
        out = np.where(idx == f1, f0, np.float32(0.0))
        acc = out.sum(axis=1, dtype=np.float64).astype(np.float32)[:, None]
        return out.reshape(in0.shape), acc

    gather_spec = Spec(
        body=select(eq(Idx, maxx(Src1, Zero - Src1)), Src0, Zero),
        accum=AluOp.ADD,
        accum_init=Zero,
        reference=_gather_ref,
    )
    gop = _register("GATHER_EQ_SUM_ANT2", gather_spec)

    # MISMATCH_XOR_SUM: out[k] = (in0[k] != 0) XOR (in1[k] > 0); accum = sum
    def _mm_ref(in0, in1, s0, s1, imm2):
        p = in0.shape[0]
        a = np.asarray(in0, np.float32).reshape(p, -1) != 0
        b = np.asarray(in1, np.float32).reshape(p, -1) > 0
        out = (a ^ b).astype(np.float32)
        acc = out.sum(axis=1, dtype=np.float64).astype(np.float32)[:, None]
        return out.reshape(in0.shape), acc

    mm_spec = Spec(
        body=Bin(AluOp.LOGICAL_XOR, Src0, Src1 > Zero),
        accum=AluOp.ADD,
        accum_init=Zero,
        reference=_mm_ref,
    )
    mop = _register("MISMATCH_XOR_SUM_ANT2", mm_spec)
    return gop, mop


# ------------------------------------------------------------- device build
def _build_nc(tiles=TILES, w=W):
    import concourse.bass as bass
    import concourse.tile as tile
    from concourse import bacc, mybir

    gop, mop = _register_custom_ops()
    f32 = mybir.dt.float32
    A = mybir.ActivationFunctionType
    X = mybir.AxisListType.X
    XY = mybir.AxisListType.XY
    alu = mybir.AluOpType

    nc = bacc.Bacc("TRN2", target_bir_lowering=False, debug=False,
                   num_devices=N_CORES)
    pred_d = nc.dram_tensor("pred", [tiles, P, w * C], f32,
                            kind="ExternalInput").ap()
    gs_d = nc.dram_tensor("gs", [tiles, P, w], f32,
                          kind="ExternalInput").ap()
    out_d = nc.dram_tensor("out", [P, 3], f32, kind="ExternalOutput").ap()

    with tile.TileContext(nc) as tc:
        with (
            tc.tile_pool(name="io", bufs=2) as io,
            tc.tile_pool(name="ep", bufs=2) as ep,
            tc.tile_pool(name="zp", bufs=2) as zp,
            tc.tile_pool(name="mp", bufs=2) as mp,
            tc.tile_pool(name="tp", bufs=1) as tp,
            tc.tile_pool(name="cp", bufs=1) as cp,
        ):
            bias16 = cp.tile([P, 1], f32)
            nc.gpsimd.memset(bias16[:], -SHIFT)
            acc_lg = cp.tile([P, tiles], f32)
            acc_g = cp.tile([P, tiles], f32)
            acc_mm = cp.tile([P, tiles], f32)
            trash = tp.tile([P, w * C], f32)
            trash2 = tp.tile([P, w], f32)

            for i in range(tiles):
                pt = io.tile([P, w * C], f32, tag="pred")
                nc.sync.dma_start(pt[:], pred_d[i])
                gt = io.tile([P, w], f32, tag="gs")
                nc.sync.dma_start(gt[:], gs_d[i])

                # ---- CE path: exp on ACT, row-sum tree on GPSIMD, ln on ACT
                et = ep.tile([P, w * C], f32, tag="E")
                nc.scalar.activation(et[:], pt[:], A.Exp, bias=bias16[:])

                e3 = et[:].rearrange("p (w c) -> p w c", c=C)
                z1 = zp.tile([P, w, 5], f32, tag="z1")
                nc.gpsimd.tensor_tensor(z1[:], e3[:, :, 0:5], e3[:, :, 5:10],
                                        op=alu.add)
                z2 = zp.tile([P, w, 2], f32, tag="z2")
                nc.gpsimd.tensor_tensor(z2[:], z1[:, :, 0:2], z1[:, :, 2:4],
                                        op=alu.add)
                z3 = zp.tile([P, w], f32, tag="z3")
                nc.gpsimd.tensor_tensor(z3[:], z2[:, :, 0], z2[:, :, 1],
                                        op=alu.add)
                s = zp.tile([P, w], f32, tag="s")
                nc.gpsimd.tensor_tensor(s[:], z3[:], z1[:, :, 4], op=alu.add)

                lg = zp.tile([P, w], f32, tag="lg")
                nc.scalar.activation(lg[:], s[:], A.Ln,
                                     accum_out=acc_lg[:, i:i + 1])

                # ---- BCE path: group maxes on DVE
                p3 = pt[:].rearrange("p (w c) -> p w c", c=C)
                p4 = pt[:].rearrange("p (w g e) -> p w g e", g=5, e=2)
                m6 = mp.tile([P, w], f32, tag="m6")
                nc.vector.reduce_max(m6[:], p3[:, :, 2:8], axis=X)
                m4 = mp.tile([P, w], f32, tag="m4")
                nc.vector.reduce_max(m4[:], p4[:, :, 0:5:4, :], axis=XY)
                bp = mp.tile([P, w], f32, tag="bp")
                nc.vector.tensor_tensor(bp[:], m6[:], m4[:], op=alu.is_gt)

                # ---- fused gather + mismatch accumulation (DVE)
                nc.vector._custom_dve(
                    gop, out=trash[:],
                    in0=pt[:].rearrange("p (w c) -> p w c", c=C),
                    in1=gt[:].unsqueeze(2).broadcast_to([P, w, C]),
                    accum_out=acc_g[:, i:i + 1])
                nc.vector._custom_dve(
                    mop, out=trash2[:], in0=bp[:], in1=gt[:],
                    accum_out=acc_mm[:, i:i + 1])

            # ---- final per-partition reductions + store
            out_t = cp.tile([P, 3], f32)
            nc.vector.reduce_sum(out_t[:, 0:1], acc_lg[:], axis=X)
            nc.vector.reduce_sum(out_t[:, 1:2], acc_g[:], axis=X)
            nc.vector.reduce_sum(out_t[:, 2:3], acc_mm[:], axis=X)
            nc.sync.dma_start(out_d[:], out_t[:])

    # Force a single activation table containing both Exp and Ln so the
    # compiler does not ping-pong ACT_TABLE_LOADs.  Table ids are positional,
    # so keep the dict shape and empty the other sets.
    import concourse.bacc as bacc_mod
    from concourse.hw_specs import get_activation_tables
    orig = get_activation_tables(nc.m.arch)
    combined = None
    for k, v in orig.items():
        if (mybir.ActivationFunctionType.Exp in v
                and mybir.ActivationFunctionType.Ln in v):
            combined = k
            break
    if combined is not None:
        patched = {k: (v if k == combined else set()) for k, v in orig.items()}
        saved = bacc_mod.get_activation_tables
        bacc_mod.get_activation_tables = lambda arch: patched
        try:
            nc.compile()
        finally:
            bacc_mod.get_activation_tables = saved
    else:
        nc.compile()
    return nc


def _get_nc():
    if "nc" not in _CACHE:
        _CACHE["nc"] = _build_nc()
    return _CACHE["nc"]


# ------------------------------------------------------------------- host
def _host_prep(pred, target):
    """Shard + pad inputs, build the packed gs aux tensor per core."""
    pred = np.ascontiguousarray(np.asarray(pred, dtype=np.float32))
    target = np.asarray(target).astype(np.int32)

    in_maps = []
    rows = ROWS_CORE
    for c in range(N_CORES):
        pc = pred[c * rows:(c + 1) * rows] + np.float32(SHIFT)
        tc_ = target[c * rows:(c + 1) * rows]
        if PAD_PER_CORE:
            pc = np.concatenate(
                [pc, np.full((PAD_PER_CORE, C), SHIFT, np.float32)], axis=0)
            tc_ = np.concatenate(
                [tc_, np.zeros(PAD_PER_CORE, np.int32)], axis=0)
        pc = pc.reshape(TILES, P, W * C)
        tc_ = tc_.reshape(TILES, P, W)
        w_idx = np.broadcast_to(
            np.arange(W, dtype=np.int64) * C, (TILES, P, W))
        g = (w_idx + tc_).astype(np.float32)
        bt = (tc_ >= 2) & (tc_ <= 7)
        gs = np.where(bt, g, -g).astype(np.float32)
        in_maps.append({"pred": np.ascontiguousarray(pc),
                        "gs": np.ascontiguousarray(gs)})
    return in_maps


def kernel(pred, target):
    from concourse.bass_utils import run_bass_kernel_spmd

    nc = _get_nc()
    in_maps = _host_prep(pred, target)
    res = run_bass_kernel_spmd(nc, in_maps, core_ids=list(range(N_CORES)))

    sum_lg = 0.0
    sum_g = 0.0
    sum_mm = 0.0
    for c in range(N_CORES):
        o = res.results[c]["out"].astype(np.float64)
        sum_lg += o[:, 0].sum()
        sum_g += o[:, 1].sum()
        sum_mm += o[:, 2].sum()

    # padded rows: pred' = 16 -> logsumexp = ln(10), gather = 16, mismatch 0.
    # every (real and padded) row's gather picks pred + 16.
    sum_lg -= N_CORES * PAD_PER_CORE * np.log(10.0)
    sum_g -= SHIFT * N_CORES * ROWS_CORE_PAD

    ce = (sum_lg - sum_g) / N
    bce = 100.0 * sum_mm / N
    return np.float32(ce + bce)
